# revision 1
# baseline (speedup 1.0000x reference)
"""Trainium2 Bass kernel for ContinuousAttention (self-keyed RoPE attention,
strictly-causal masked scores, no softmax).

Reference computation (B=2, NH=16, T=2048, N=256, fp32):
    QR = rope(Q)                      # interleaved-pair RoPE, freqs quantized in pairs
    S  = QR @ QR^T                    # per (b, h); K input is unused by the module
    O  = (S * strict_causal_mask) @ V

Sharding: 32 (b*nh) heads over 8 NeuronCores, 4 heads per core; no
communication.  Each core runs an identical program on its head slice.

v2 design (fp16 matmul operands, fp32 PSUM accumulation):
  - Host ships Q, pair-swapped Q, and V in fp16, plus transposed RoPE tables.
  - Per head, xbar DMA-transposes Q / Qswap chunks straight from DRAM into
    (n, t) layout; RoPE is then 3 dense DVE ops per 128-partition chunk:
        QRT = QT * cosT + QswapT * sinT_signed
  - matmul1: T_ij = S_ij^T strips (stationary = QRT j-block, moving = QRT
    256-wide t-group), only causal-triangle groups; PSUM->SBUF copies cast to
    fp16 and apply the strict mask on diagonal blocks.
  - matmul2: O_i = sum_{j<=i} T_ij^T @ V_j accumulated in PSUM (fp32).
  - O tiles -> fp32 staging tile -> one DMA per head.
"""

import math
import sys

import numpy as np

if "/opt/trn_rl_repo" not in sys.path:
    sys.path.insert(0, "/opt/trn_rl_repo")

import concourse.bass as bass
import concourse.mybir as mybir
import concourse.tile as tile
from concourse.bass_utils import run_bass_kernel_spmd

B, NH, T, N = 2, 16, 2048, 256
THETA = 2 ** 16
N_CORES = 8
H_PER_CORE = (B * NH) // N_CORES

F32 = mybir.dt.float32
FP16 = mybir.dt.float16
MULT = mybir.AluOpType.mult
ADD = mybir.AluOpType.add
HF = np.float16


def _split_overloaded_waits(nc, max_waits=1):
    """walrus in this container rejects >1 sync-wait per instruction; move
    extra waits onto preceding same-engine NoOps (semantically identical)."""
    n_split = 0
    for f in nc.m.functions:
        for bb in f.blocks:
            new_list = []
            changed = False
            for ins in bb.instructions:
                si = getattr(ins, "sync_info", None)
                if si is not None and len(si.on_wait) > max_waits:
                    waits = list(si.on_wait)
                    extra, keep = waits[:-max_waits], waits[-max_waits:]
                    k = 0
                    while extra:
                        chunk, extra = extra[:max_waits], extra[max_waits:]
                        nop = mybir.InstNoOp(
                            name=f"{ins.name}_wsplit{k}", ins=[], outs=[]
                        )
                        nop.engine = ins.engine
                        nop.sync_info = mybir.SyncInfo(on_wait=chunk, on_update=[])
                        new_list.append(nop)
                        k += 1
                    ins.sync_info = mybir.SyncInfo(
                        on_wait=keep, on_update=list(si.on_update)
                    )
                    changed = True
                    n_split += 1
                new_list.append(ins)
            if changed:
                bb.instructions = new_list
    return n_split


def rope_tables(t=T, n=N, dtype=np.float32):
    """cos table and sign-folded sin table, natural (t, n) layout."""
    idx = np.floor(np.arange(n, dtype=dtype) / dtype(2.0)) * dtype(2.0)
    freqs = (
        dtype(1.0) / (dtype(THETA) ** (idx / dtype(n))) / dtype(2.0 * math.pi)
    ).astype(dtype)
    phases = np.arange(t, dtype=dtype)[:, None] * freqs[None, :]
    ph = (phases % dtype(1.0)) * dtype(2.0 * math.pi)
    cos = np.cos(ph).astype(dtype)
    sin = np.sin(ph).astype(dtype)
    sin_a = sin.copy()
    sin_a[:, 0::2] *= dtype(-1.0)  # fold the rotate-pair sign into sin
    return cos, sin_a


def build_nc(h_per_core=H_PER_CORE, t=T, n=N, waitsplit=True):
    """v4: RoPE precomputed on host (ships QR directly) — the device does
    only the two chained matmuls.  mm1 runs trimmed-causal in <=512-wide
    moving chunks (80 matmuls/head, zero wasted blocks); mm2 for a 512-row
    t-chunk is interleaved right after that chunk's mm1 so PSUM-drain
    pressure on Vector/Scalar stays smooth and the PE never starves."""
    assert n == 256 and t % 512 == 0
    nt = t // 128   # 128-row s-blocks (16)
    ntc = t // 512  # 512-wide t-chunks (4)
    nc = bass.Bass("TRN2", target_bir_lowering=False, debug=False)

    qrtd = nc.dram_tensor("qrt", [h_per_core, n, t], FP16, kind="ExternalInput").ap()
    v = nc.dram_tensor("v", [h_per_core, t, n], FP16, kind="ExternalInput").ap()
    o = nc.dram_tensor("o", [h_per_core, t, n], F32, kind="ExternalOutput").ap()

    with tile.TileContext(nc) as tc:
        with (
            tc.tile_pool(name="const", bufs=1) as cpool,
            tc.tile_pool(name="qrt", bufs=2) as qrtpool,
            tc.tile_pool(name="strips", bufs=2) as strippool,
            tc.tile_pool(name="vh", bufs=2) as vpool,
            tc.tile_pool(name="oh", bufs=2) as ohpool,
            tc.tile_pool(name="sps", bufs=5, space="PSUM") as spool,
            tc.tile_pool(name="ops", bufs=3, space="PSUM") as opool,
        ):
            # diag-chunk mask, (s, t) orientation: cols<128 keep iff t>s,
            # cols>=128 always keep (t provably > s there)
            mask = cpool.tile([128, 512], F32)
            nc.gpsimd.memset(mask, 1.0)
            nc.gpsimd.affine_select(
                out=mask[:, 0:128],
                in_=mask[:, 0:128],
                compare_op=mybir.AluOpType.is_ge,
                fill=0.0,
                base=-1,
                pattern=[[1, 128]],
                channel_multiplier=-1,
            )

            # HAM warmup: dummy PE activity while head 0's input DMAs are in
            # flight starts the un-throttle clock early; sized so it ends
            # about when the first qrt chunk lands (a full 3.4us warmup would
            # delay real work more than the cold tax it avoids).
            warm = spool.tile([128, 512], F32, tag="ps", name="warm")
            for _ in range(2):  # fp32 = 4 cyc/row -> ~1.7us each cold
                nc.tensor.matmul(
                    warm, lhsT=mask[:, 0:128], rhs=mask, start=True, stop=True
                )

            dr = 0  # full-chunk drain round robin (scalar / vector)
            do = 0  # O drain round robin
            for h in range(h_per_core):
                # ---- DMA rotated Q, (n, t) layout, 512-col segments ----
                qrt = [
                    qrtpool.tile([128, t], FP16, tag=f"qrt{c}", name=f"qrt{c}")
                    for c in range(2)
                ]
                for s in range(ntc):
                    tsl = slice(s * 512, (s + 1) * 512)
                    for c in range(2):
                        psl = slice(c * 128, (c + 1) * 128)
                        nc.sync.dma_start(
                            out=qrt[c][:, tsl], in_=qrtd[h][psl, tsl]
                        )
                # vh on scalar: one descriptor ahead of the drain copies;
                # needed only once mm2 of t-chunk 0 starts
                vh = vpool.tile([128, nt * n], FP16, tag="vh", name="vh")
                nc.scalar.dma_start(
                    out=vh.rearrange("p (t n) -> p t n", n=n),
                    in_=v[h].rearrange("(t p) n -> p t n", p=128),
                )

                strips = [
                    strippool.tile(
                        [128, t - 128 * j], FP16,
                        tag=f"strip{j}", name=f"strip{j}",
                    )
                    for j in range(nt)
                ]
                oh = ohpool.tile([128, nt * n], F32, tag="oh", name="oh")

                def mm2(po, i, j):
                    loff = 128 * (i - j)
                    nc.tensor.matmul(
                        po,
                        lhsT=strips[j][:, loff:loff + 128],
                        rhs=vh[:, j * n:(j + 1) * n],
                        start=(j == 0),
                        stop=(j == i),
                    )

                def o_writeback(i, po):
                    nonlocal do
                    dst = oh[:, i * n:(i + 1) * n]
                    if do % 2 == 0:
                        nc.vector.tensor_copy(out=dst, in_=po)
                    else:
                        nc.scalar.copy(out=dst, in_=po)
                    do += 1

                def emit_mm1(tcx):
                    nonlocal dr
                    base_t = 512 * tcx
                    for j in range(4 * tcx + 4):
                        diag = j >= 4 * tcx
                        off = 128 * (j - 4 * tcx) if diag else 0
                        width = 512 - off
                        col0 = base_t + off
                        ps = spool.tile([128, 512], F32, name="ps")
                        for c in range(2):
                            nc.tensor.matmul(
                                ps[:, :width],
                                lhsT=qrt[c][:, j * 128:(j + 1) * 128],
                                rhs=qrt[c][:, col0:base_t + 512],
                                start=(c == 0),
                                stop=(c == 1),
                            )
                        dst = strips[j][:, col0 - 128 * j:col0 - 128 * j + width]
                        if diag:  # fused strict-causal mask + cast drain
                            nc.vector.tensor_tensor(
                                out=dst, in0=ps[:, :width],
                                in1=mask[:, :width], op=MULT,
                            )
                        else:
                            if dr % 2 == 0:
                                nc.scalar.copy(out=dst, in_=ps[:, :width])
                            else:
                                nc.vector.tensor_copy(out=dst, in_=ps[:, :width])
                            dr += 1

                def emit_mm2(tcx, last=False):
                    pairs = list(range(4 * tcx, 4 * tcx + 4, 2))
                    if last:
                        # longest accumulation chains first so the kernel
                        # tail ends on the shortest one
                        pairs = pairs[::-1]
                    for i0 in pairs:
                        i1 = i0 + 1
                        po0 = opool.tile([128, n], F32, name="po")
                        po1 = opool.tile([128, n], F32, name="po")
                        for j in range(i0 + 1):
                            mm2(po0, i0, j)
                            mm2(po1, i1, j)
                        mm2(po1, i1, i1)
                        o_writeback(i0, po0)
                        o_writeback(i1, po1)
                        # drain output per i-pair: keeps the final DMA small
                        # so the kernel tail is short
                        isl = slice(i0 * 128, (i1 + 1) * 128)
                        csl = slice(i0 * n, (i1 + 1) * n)
                        nc.sync.dma_start(
                            out=o[h][isl].rearrange("(t p) n -> p t n", p=128),
                            in_=oh[:, csl].rearrange("p (t n) -> p t n", n=n),
                        )

                # mm1 runs one t-chunk ahead of mm2 so the PE always has
                # matmul work queued while a chunk's PSUM drains complete
                emit_mm1(0)
                for tcx in range(ntc):
                    if tcx + 1 < ntc:
                        emit_mm1(tcx + 1)
                    emit_mm2(
                        tcx, last=(h == h_per_core - 1 and tcx == ntc - 1)
                    )

    if waitsplit:
        _split_overloaded_waits(nc)
    return nc


_NC_CACHE = {}


def get_nc(h_per_core=H_PER_CORE, t=T, n=N):
    key = (h_per_core, t, n)
    if key not in _NC_CACHE:
        _NC_CACHE[key] = build_nc(h_per_core, t, n)
    return _NC_CACHE[key]


def make_in_maps(Q, V, n_cores=N_CORES):
    b, nh, t, n = Q.shape
    h_per_core = (b * nh) // n_cores
    qf = np.asarray(Q, dtype=np.float32).reshape(b * nh, t, n)
    vf = np.asarray(V, dtype=np.float32).reshape(b * nh, t, n)
    # RoPE on host in fp32 (input prep, like the layout transposes):
    # qr = q * cos + pairswap(q) * sign-folded-sin
    qsw = qf.reshape(b * nh, t, n // 2, 2)[..., ::-1].reshape(b * nh, t, n)
    cos, sin_a = rope_tables(t, n)
    qr = (qf * cos + qsw * sin_a).astype(HF)
    # pre-transposed (n, t) layout so the device needs only plain DMAs
    qrtb = np.ascontiguousarray(qr.transpose(0, 2, 1))
    vb = vf.astype(HF)
    in_maps = []
    for c in range(n_cores):
        sl = slice(c * h_per_core, (c + 1) * h_per_core)
        in_maps.append(
            {
                "qrt": np.ascontiguousarray(qrtb[sl]),
                "v": np.ascontiguousarray(vb[sl]),
            }
        )
    return in_maps


def kernel(Q, K, V):
    """Full-input entry point: Q, K, V are (B, NH, T, N) float32 numpy arrays.
    K is unused (the module self-keys attention on rotated Q)."""
    Q = np.asarray(Q)
    V = np.asarray(V)
    b, nh, t, n = Q.shape
    nc = get_nc((b * nh) // N_CORES, t, n)
    in_maps = make_in_maps(Q, V, N_CORES)
    res = None
    last_err = None
    for attempt in range(3):  # retry transient device/runtime failures
        try:
            res = run_bass_kernel_spmd(
                nc, in_maps, core_ids=list(range(N_CORES)), trace=False
            )
            break
        except Exception as e:  # e.g. NRT_EXEC_UNIT_UNRECOVERABLE after a
            last_err = e  # wedged prior run; a clean retry usually recovers
            import time as _time

            _time.sleep(2.0 * (attempt + 1))
    if res is None:
        raise last_err
    outs = [res.results[c]["o"] for c in range(N_CORES)]
    out = np.concatenate(outs, axis=0).reshape(b, nh, t, n)
    return out.astype(np.float32)



# revision 12
# speedup vs baseline: 1.0376x; 1.0376x over previous
"""Trainium2 Bass kernel for ContinuousAttention (self-keyed RoPE attention,
strictly-causal masked scores, no softmax).

Reference computation (B=2, NH=16, T=2048, N=256, fp32):
    QR = rope(Q)                      # interleaved-pair RoPE, freqs quantized in pairs
    S  = QR @ QR^T                    # per (b, h); K input is unused by the module
    O  = (S * strict_causal_mask) @ V

Sharding: 32 (b*nh) heads over 8 NeuronCores, 4 heads per core; no
communication.  Each core runs an identical program on its head slice.

v5 design — chunked linear attention (no softmax => scores are linear):
    O_i = QR_i @ H_{<i} + tril_strict(QR_i QR_i^T) @ V_i,   H_i = QR_i^T V_i
with a running state H (256x256) accumulated in fp32 PSUM across the 16
128-row chunks of each head.  PE work drops from ~T^2*N (dense causal) to
~2*T*N^2 + 2*T*C*N per head (2.8x less).

Per chunk (C=128): PE transposes the QR chunk (state lhsT), intra1 computes
the 128x128 diagonal score block, inter applies H_{<i}, intra2 applies the
strict-masked diagonal block, state accumulates H += QR_i^T V_i.  Drains
(mask-mult, H fp32->fp16 copy, O cast) are spread over vector/scalar/gpsimd.
Two heads are interleaved chunk-by-chunk so every drain has a full other-head
chunk (~1536 PE cols) of latency cover.

Host ships QR pre-rotated in (n, t) layout fp16 and V p-major-packed fp16;
all device DMAs are plain contiguous 2D copies.  Output is written fp16
p-major and unpacked on host.
"""

import math
import sys

import numpy as np

if "/opt/trn_rl_repo" not in sys.path:
    sys.path.insert(0, "/opt/trn_rl_repo")

import concourse.bass as bass
import concourse.mybir as mybir
import concourse.tile as tile
from concourse import masks
from concourse.bass_utils import run_bass_kernel_spmd

B, NH, T, N = 2, 16, 2048, 256
THETA = 2 ** 16
N_CORES = 8
H_PER_CORE = (B * NH) // N_CORES

F32 = mybir.dt.float32
FP16 = mybir.dt.float16
MULT = mybir.AluOpType.mult
HF = np.float16


def _split_overloaded_waits(nc, max_waits=1):
    """walrus in this container rejects >1 sync-wait per instruction; move
    extra waits onto preceding same-engine NoOps (semantically identical)."""
    n_split = 0
    for f in nc.m.functions:
        for bb in f.blocks:
            new_list = []
            changed = False
            for ins in bb.instructions:
                si = getattr(ins, "sync_info", None)
                if si is not None and len(si.on_wait) > max_waits:
                    waits = list(si.on_wait)
                    extra, keep = waits[:-max_waits], waits[-max_waits:]
                    k = 0
                    while extra:
                        chunk, extra = extra[:max_waits], extra[max_waits:]
                        nop = mybir.InstNoOp(
                            name=f"{ins.name}_wsplit{k}", ins=[], outs=[]
                        )
                        nop.engine = ins.engine
                        nop.sync_info = mybir.SyncInfo(on_wait=chunk, on_update=[])
                        new_list.append(nop)
                        k += 1
                    ins.sync_info = mybir.SyncInfo(
                        on_wait=keep, on_update=list(si.on_update)
                    )
                    changed = True
                    n_split += 1
                new_list.append(ins)
            if changed:
                bb.instructions = new_list
    return n_split


def rope_tables(t=T, n=N, dtype=np.float32):
    """cos table and sign-folded sin table, natural (t, n) layout."""
    idx = np.floor(np.arange(n, dtype=dtype) / dtype(2.0)) * dtype(2.0)
    freqs = (
        dtype(1.0) / (dtype(THETA) ** (idx / dtype(n))) / dtype(2.0 * math.pi)
    ).astype(dtype)
    phases = np.arange(t, dtype=dtype)[:, None] * freqs[None, :]
    ph = (phases % dtype(1.0)) * dtype(2.0 * math.pi)
    cos = np.cos(ph).astype(dtype)
    sin = np.sin(ph).astype(dtype)
    sin_a = sin.copy()
    sin_a[:, 0::2] *= dtype(-1.0)  # fold the rotate-pair sign into sin
    return cos, sin_a


def build_nc(h_per_core=H_PER_CORE, t=T, n=N, waitsplit=True):
    assert n == 256 and t % 128 == 0
    nt = t // 128  # 128-row chunks per head (16)
    nc = bass.Bass("TRN2", target_bir_lowering=False, debug=False)

    # qrt: rotated Q, (n, t) layout, split into two 128-partition n-halves
    qrtd = nc.dram_tensor(
        "qrt", [h_per_core, 2, 128, t], FP16, kind="ExternalInput"
    ).ap()
    # v: p-major packed: v[h, p, i*n + m] = V[h, i*128 + p, m]
    vd = nc.dram_tensor(
        "v", [h_per_core, 128, nt * n], FP16, kind="ExternalInput"
    ).ap()
    # o: same p-major packing, fp16; host unpacks + casts
    od = nc.dram_tensor(
        "o", [h_per_core, 128, nt * n], FP16, kind="ExternalOutput"
    ).ap()

    with tile.TileContext(nc) as tc:
        with (
            tc.tile_pool(name="const", bufs=1) as cpool,
            tc.tile_pool(name="qrt", bufs=4) as qpool,
            tc.tile_pool(name="vh", bufs=4) as vpool,
            tc.tile_pool(name="hs", bufs=2) as hspool,
            tc.tile_pool(name="sts", bufs=4) as stspool,
            tc.tile_pool(name="tns", bufs=4) as tnspool,
            tc.tile_pool(name="ohs", bufs=4) as ohpool,
            tc.tile_pool(name="hp", bufs=2, space="PSUM") as hpool,
            tc.tile_pool(name="op", bufs=2, space="PSUM") as opool,
            tc.tile_pool(name="stp", bufs=2, space="PSUM") as stpool,
            tc.tile_pool(name="tpp", bufs=1, space="PSUM") as tppool,
        ):
            # mask, (s, t) orientation: keep iff t > s (strict causal for the
            # diagonal block).  Wider than 128 so warmup matmuls can use it.
            mask = cpool.tile([128, 512], F32)
            nc.gpsimd.memset(mask, 1.0)
            nc.gpsimd.affine_select(
                out=mask[:, 0:128],
                in_=mask[:, 0:128],
                compare_op=mybir.AluOpType.is_ge,
                fill=0.0,
                base=-1,
                pattern=[[1, 128]],
                channel_multiplier=-1,
            )
            ident = cpool.tile([128, 128], FP16)
            masks.make_identity(nc, ident)

            # HAM warmup: dummy fp32 PE activity while head 0's input DMAs are
            # in flight starts the un-throttle clock early.
            for _ in range(2):
                warm = opool.tile([128, 256], F32, tag="op", name="warm")
                nc.tensor.matmul(
                    warm, lhsT=mask[:, 0:128], rhs=mask[:, 0:256],
                    start=True, stop=True,
                )

            qrt = {}
            vh = {}
            hp = {}
            hs = {}
            dr = [0, 0, 0]  # drain round-robins: o-cast, h-copy, tn/sts

            def emit_loads(h):
                qrt[h] = [
                    qpool.tile([128, t], FP16, tag=f"qrt{c}", name=f"qrt{c}_{h}")
                    for c in range(2)
                ]
                for c in range(2):
                    for s in range(4):
                        tsl = slice(s * (t // 4), (s + 1) * (t // 4))
                        nc.sync.dma_start(
                            out=qrt[h][c][:, tsl], in_=qrtd[h, c][:, tsl]
                        )
                vh[h] = vpool.tile([128, nt * n], FP16, tag="vh", name=f"vh{h}")
                for s in range(4):
                    vsl = slice(s * (nt * n // 4), (s + 1) * (nt * n // 4))
                    nc.scalar.dma_start(out=vh[h][:, vsl], in_=vd[h][:, vsl])

            def emit_transpose(h, ci):
                """qtn chunk = (QR_i)^T^T: PE-transpose the two qrt n-halves
                into (t, n) layout for the state matmul's lhsT."""
                csl = slice(ci * 128, (ci + 1) * 128)
                tp0 = tppool.tile([128, 128], FP16, name="tp0")
                tp1 = tppool.tile([128, 128], FP16, name="tp1")
                nc.tensor.transpose(tp0, qrt[h][0][:, csl], ident)
                nc.tensor.transpose(tp1, qrt[h][1][:, csl], ident)
                tn = tnspool.tile([128, 256], FP16, name="tn")
                nc.vector.tensor_copy(out=tn[:, 0:128], in_=tp0)
                nc.vector.tensor_copy(out=tn[:, 128:256], in_=tp1)
                return tn

            def emit_intra1(h, ci):
                csl = slice(ci * 128, (ci + 1) * 128)
                stp = stpool.tile([128, 128], F32, name="stp")
                nc.tensor.matmul(
                    stp, lhsT=qrt[h][0][:, csl], rhs=qrt[h][0][:, csl],
                    start=True, stop=False,
                )
                nc.tensor.matmul(
                    stp, lhsT=qrt[h][1][:, csl], rhs=qrt[h][1][:, csl],
                    start=False, stop=True,
                )
                # fused strict-causal mask + fp16 cast drain
                sts = stspool.tile([128, 128], FP16, name="sts")
                nc.vector.tensor_tensor(
                    out=sts, in0=stp, in1=mask[:, 0:128], op=MULT
                )
                return sts

            def emit_out(h, ci, sts, tn):
                """inter + intra2 (-> O chunk, drained + DMA'd) and state."""
                csl = slice(ci * 128, (ci + 1) * 128)
                msl = slice(ci * n, (ci + 1) * n)
                op = opool.tile([128, 256], F32, name="op", tag="op")
                if ci > 0:
                    nc.tensor.matmul(
                        op, lhsT=qrt[h][0][:, csl], rhs=hs[h][:, 0:256],
                        start=True, stop=False,
                    )
                    nc.tensor.matmul(
                        op, lhsT=qrt[h][1][:, csl], rhs=hs[h][:, 256:512],
                        start=False, stop=False,
                    )
                nc.tensor.matmul(
                    op, lhsT=sts, rhs=vh[h][:, msl],
                    start=(ci == 0), stop=True,
                )
                oh = ohpool.tile([128, 256], FP16, name="oh")
                if dr[0] % 2 == 0:
                    nc.scalar.copy(out=oh, in_=op)
                else:
                    nc.vector.tensor_copy(out=oh, in_=op)
                dr[0] += 1
                nc.gpsimd.dma_start(out=od[h][:, msl], in_=oh)
                if ci < nt - 1:
                    # state: H += QR_i^T V_i (fp32 PSUM accumulation spanning
                    # the whole head; reads between accumulating matmuls are
                    # fine on HW, skip the sim's group check)
                    # both H halves share one PSUM bank; start=True clears the
                    # has_written bits BANK-wide, so only the very first
                    # matmul of the head may use it.  The second half's first
                    # write relies on cleared bits => overwrite-where-unset.
                    nc.tensor.matmul(
                        hp[h][:, 0:256], lhsT=tn[:, 0:128], rhs=vh[h][:, msl],
                        start=(ci == 0), stop=(ci == nt - 2),
                        skip_group_check=True,
                    )
                    nc.tensor.matmul(
                        hp[h][:, 256:512], lhsT=tn[:, 128:256],
                        rhs=vh[h][:, msl],
                        start=False, stop=(ci == nt - 2),
                        skip_group_check=True,
                    )
                    # running H -> fp16 SBUF for next chunk's inter
                    if dr[1] % 2 == 0:
                        nc.vector.tensor_copy(out=hs[h], in_=hp[h])
                    else:
                        nc.scalar.copy(out=hs[h], in_=hp[h])
                    dr[1] += 1

            for pair in range(h_per_core // 2):
                heads = (2 * pair, 2 * pair + 1)
                for h in heads:
                    emit_loads(h)
                    hp[h] = hpool.tile([128, 512], F32, tag="hp", name=f"hp{h}")
                    hs[h] = hspool.tile([128, 512], FP16, tag="hs", name=f"hs{h}")
                # chunk 0: batch both heads' intra1 first so neither head's
                # intra2 waits on its own mask-drain with no PE cover
                work0 = {}
                for h in heads:
                    tn = emit_transpose(h, 0)
                    sts = emit_intra1(h, 0)
                    work0[h] = (sts, tn)
                for h in heads:
                    emit_out(h, 0, *work0[h])
                for ci in range(1, nt):
                    for h in heads:
                        tn = emit_transpose(h, ci) if ci < nt - 1 else None
                        sts = emit_intra1(h, ci)
                        emit_out(h, ci, sts, tn)

    if waitsplit:
        _split_overloaded_waits(nc)
    return nc


_NC_CACHE = {}


def get_nc(h_per_core=H_PER_CORE, t=T, n=N):
    key = (h_per_core, t, n)
    if key not in _NC_CACHE:
        _NC_CACHE[key] = build_nc(h_per_core, t, n)
    return _NC_CACHE[key]


def make_in_maps(Q, V, n_cores=N_CORES):
    b, nh, t, n = Q.shape
    h_per_core = (b * nh) // n_cores
    nt = t // 128
    qf = np.asarray(Q, dtype=np.float32).reshape(b * nh, t, n)
    vf = np.asarray(V, dtype=np.float32).reshape(b * nh, t, n)
    # RoPE on host in fp32 (input prep, like the layout transposes):
    # qr = q * cos + pairswap(q) * sign-folded-sin
    qsw = qf.reshape(b * nh, t, n // 2, 2)[..., ::-1].reshape(b * nh, t, n)
    cos, sin_a = rope_tables(t, n)
    qr = (qf * cos + qsw * sin_a).astype(HF)
    # (n, t) layout, n-halves split for direct 128-partition DMAs
    qrtb = np.ascontiguousarray(
        qr.transpose(0, 2, 1).reshape(b * nh, 2, 128, t)
    )
    # V p-major packed: vb[h, p, i*n + m] = V[h, i*128 + p, m]
    vb = np.ascontiguousarray(
        vf.astype(HF).reshape(b * nh, nt, 128, n).transpose(0, 2, 1, 3)
    ).reshape(b * nh, 128, nt * n)
    in_maps = []
    for c in range(n_cores):
        sl = slice(c * h_per_core, (c + 1) * h_per_core)
        in_maps.append(
            {
                "qrt": np.ascontiguousarray(qrtb[sl]),
                "v": np.ascontiguousarray(vb[sl]),
            }
        )
    return in_maps


def unpack_out(outs, b, nh, t, n):
    """[cores][h, 128, nt*n] p-major fp16 -> (b, nh, t, n) fp32."""
    nt = t // 128
    full = np.concatenate(outs, axis=0)  # (b*nh, 128, nt*n)
    full = full.reshape(b * nh, 128, nt, n).transpose(0, 2, 1, 3)
    return np.ascontiguousarray(full).reshape(b, nh, t, n).astype(np.float32)


def kernel(Q, K, V):
    """Full-input entry point: Q, K, V are (B, NH, T, N) float32 numpy arrays.
    K is unused (the module self-keys attention on rotated Q)."""
    Q = np.asarray(Q)
    V = np.asarray(V)
    b, nh, t, n = Q.shape
    nc = get_nc((b * nh) // N_CORES, t, n)
    in_maps = make_in_maps(Q, V, N_CORES)
    res = None
    last_err = None
    for attempt in range(3):  # retry transient device/runtime failures
        try:
            res = run_bass_kernel_spmd(
                nc, in_maps, core_ids=list(range(N_CORES)), trace=False
            )
            break
        except Exception as e:  # e.g. NRT_EXEC_UNIT_UNRECOVERABLE after a
            last_err = e  # wedged prior run; a clean retry usually recovers
            import time as _time

            _time.sleep(2.0 * (attempt + 1))
    if res is None:
        raise last_err
    outs = [res.results[c]["o"] for c in range(N_CORES)]
    return unpack_out(outs, b, nh, t, n)


# revision 14
# speedup vs baseline: 1.4287x; 1.3770x over previous
"""Trainium2 Bass kernel for ContinuousAttention (self-keyed RoPE attention,
strictly-causal masked scores, no softmax).

Reference computation (B=2, NH=16, T=2048, N=256, fp32):
    QR = rope(Q)                      # interleaved-pair RoPE, freqs quantized in pairs
    S  = QR @ QR^T                    # per (b, h); K input is unused by the module
    O  = (S * strict_causal_mask) @ V

Sharding: 32 (b*nh) heads over 8 NeuronCores, 4 heads per core; no
communication.  Each core runs an identical program on its head slice.

v6 design — chunked linear attention (no softmax => scores are linear):
    O_i = QR_i @ H_{<i} + (causal diagonal blocks) @ V,   H += QR_i^T V_i
with a running state H (256x256) accumulated in fp32 PSUM across each head.
PE work is ~2*T*N^2 + ~2.5*T*C*N per head, ~2.7x less than dense-causal.

Superchunks of 256 rows (2 chunks i0, i1) keep the PSUM-drain op count low
(vector/scalar are the only engines that may read PSUM and each drain op has
a few-hundred-ns fixed cost):
  - one [128, 384] score PSUM bank holds diag(i0) | dense(i1,i0) | diag(i1),
    drained by a single mask-multiply-cast (mask = strict|ones|strict),
  - one [128, 512] O PSUM bank holds O_i0 | O_i1, drained by a single cast,
  - one H copy per superchunk; O_i1's missing chunk-i0 term comes from the
    dense block instead of H.
PSUM has_written semantics: start=True clears the accumulate bits of the
WHOLE bank, so only the first matmul targeting a bank uses it; later groups
in the same bank open with start=False (overwrite-where-unset).

Host ships QR pre-rotated in both (n, t) and p-major (t, n) fp16 layouts and
V p-major fp16; all device DMAs are contiguous 2D copies.  Output is fp16
p-major, unpacked on host.  Two heads are interleaved superchunk-by-
superchunk so every drain has a full other-head superchunk of latency cover.
"""

import math
import sys

import numpy as np

if "/opt/trn_rl_repo" not in sys.path:
    sys.path.insert(0, "/opt/trn_rl_repo")

import concourse.bass as bass
import concourse.mybir as mybir
import concourse.tile as tile
from concourse.bass_utils import run_bass_kernel_spmd

B, NH, T, N = 2, 16, 2048, 256
THETA = 2 ** 16
N_CORES = 8
H_PER_CORE = (B * NH) // N_CORES

F32 = mybir.dt.float32
FP16 = mybir.dt.float16
MULT = mybir.AluOpType.mult
HF = np.float16


def _split_overloaded_waits(nc, max_waits=1):
    """walrus in this container rejects >1 sync-wait per instruction; move
    extra waits onto preceding same-engine NoOps (semantically identical)."""
    n_split = 0
    for f in nc.m.functions:
        for bb in f.blocks:
            new_list = []
            changed = False
            for ins in bb.instructions:
                si = getattr(ins, "sync_info", None)
                if si is not None and len(si.on_wait) > max_waits:
                    waits = list(si.on_wait)
                    extra, keep = waits[:-max_waits], waits[-max_waits:]
                    k = 0
                    while extra:
                        chunk, extra = extra[:max_waits], extra[max_waits:]
                        nop = mybir.InstNoOp(
                            name=f"{ins.name}_wsplit{k}", ins=[], outs=[]
                        )
                        nop.engine = ins.engine
                        nop.sync_info = mybir.SyncInfo(on_wait=chunk, on_update=[])
                        new_list.append(nop)
                        k += 1
                    ins.sync_info = mybir.SyncInfo(
                        on_wait=keep, on_update=list(si.on_update)
                    )
                    changed = True
                    n_split += 1
                new_list.append(ins)
            if changed:
                bb.instructions = new_list
    return n_split


def rope_tables(t=T, n=N, dtype=np.float32):
    """cos table and sign-folded sin table, natural (t, n) layout."""
    idx = np.floor(np.arange(n, dtype=dtype) / dtype(2.0)) * dtype(2.0)
    freqs = (
        dtype(1.0) / (dtype(THETA) ** (idx / dtype(n))) / dtype(2.0 * math.pi)
    ).astype(dtype)
    phases = np.arange(t, dtype=dtype)[:, None] * freqs[None, :]
    ph = (phases % dtype(1.0)) * dtype(2.0 * math.pi)
    cos = np.cos(ph).astype(dtype)
    sin = np.sin(ph).astype(dtype)
    sin_a = sin.copy()
    sin_a[:, 0::2] *= dtype(-1.0)  # fold the rotate-pair sign into sin
    return cos, sin_a


def build_nc(h_per_core=H_PER_CORE, t=T, n=N, waitsplit=True):
    assert n == 256 and t % 256 == 0
    nt = t // 128   # 128-row chunks per head (16)
    ns = t // 256   # superchunks per head (8)
    nc = bass.Bass("TRN2", target_bir_lowering=False, debug=False)

    # qrt: rotated Q, (n, t) layout, two 128-partition n-halves
    qrtd = nc.dram_tensor(
        "qrt", [h_per_core, 2, 128, t], FP16, kind="ExternalInput"
    ).ap()
    # qtn: rotated Q, p-major packed (t, n): qtn[h, p, ci*n+m] = QR[h, ci*128+p, m]
    qtnd = nc.dram_tensor(
        "qtn", [h_per_core, 128, nt * n], FP16, kind="ExternalInput"
    ).ap()
    # v: p-major packed like qtn
    vd = nc.dram_tensor(
        "v", [h_per_core, 128, nt * n], FP16, kind="ExternalInput"
    ).ap()
    # o: p-major packed fp16; host unpacks + casts
    od = nc.dram_tensor(
        "o", [h_per_core, 128, nt * n], FP16, kind="ExternalOutput"
    ).ap()

    with tile.TileContext(nc) as tc:
        with (
            tc.tile_pool(name="const", bufs=1) as cpool,
            tc.tile_pool(name="qrt", bufs=4) as qpool,
            tc.tile_pool(name="qtn", bufs=4) as qnpool,
            tc.tile_pool(name="vh", bufs=4) as vpool,
            tc.tile_pool(name="hs", bufs=2) as hspool,
            tc.tile_pool(name="sts", bufs=3) as stspool,
            tc.tile_pool(name="ohs", bufs=3) as ohpool,
            tc.tile_pool(name="hp", bufs=2, space="PSUM") as hpool,
            tc.tile_pool(name="op", bufs=3, space="PSUM") as opool,
            tc.tile_pool(name="sp", bufs=3, space="PSUM") as sppool,
        ):
            # mask for one superchunk's score drain, (s, t') orientation:
            # [0:128]  = strict upper (diag i0), [128:256] = ones (dense),
            # [256:384]= strict upper (diag i1), [384:512] = ones (warmup).
            mask = cpool.tile([128, 512], F32)
            nc.gpsimd.memset(mask, 1.0)
            for c0 in (0, 256):
                nc.gpsimd.affine_select(
                    out=mask[:, c0:c0 + 128],
                    in_=mask[:, c0:c0 + 128],
                    compare_op=mybir.AluOpType.is_ge,
                    fill=0.0,
                    base=-1,
                    pattern=[[1, 128]],
                    channel_multiplier=-1,
                )

            # HAM warmup: dummy fp32 PE activity while head 0's input DMAs
            # are in flight starts the un-throttle clock early.
            for _ in range(2):
                warm = opool.tile([128, 512], F32, tag="op", name="warm")
                nc.tensor.matmul(
                    warm, lhsT=mask[:, 0:128], rhs=mask,
                    start=True, stop=True,
                )

            qrt = {}
            qtn = {}
            vh = {}
            hp = {}
            hs = {}
            dr = [0]

            def emit_loads(h):
                qrt[h] = [
                    qpool.tile([128, t], FP16, tag=f"qrt{c}", name=f"qrt{c}_{h}")
                    for c in range(2)
                ]
                for c in range(2):
                    for s in range(4):
                        tsl = slice(s * (t // 4), (s + 1) * (t // 4))
                        nc.sync.dma_start(
                            out=qrt[h][c][:, tsl], in_=qrtd[h, c][:, tsl]
                        )
                qtn[h] = qnpool.tile([128, nt * n], FP16, tag="qtn", name=f"qtn{h}")
                vh[h] = vpool.tile([128, nt * n], FP16, tag="vh", name=f"vh{h}")
                for s in range(4):
                    vsl = slice(s * (nt * n // 4), (s + 1) * (nt * n // 4))
                    nc.scalar.dma_start(out=vh[h][:, vsl], in_=vd[h][:, vsl])
                    qeng = nc.sync if s % 2 == 0 else nc.scalar
                    qeng.dma_start(out=qtn[h][:, vsl], in_=qtnd[h][:, vsl])

            def emit_scores(h, sc):
                """diag(i0) | dense(i1<-i0) | diag(i1) into one PSUM bank,
                single fused mask-mult-cast drain."""
                i0s = slice(sc * 256, sc * 256 + 128)
                i1s = slice(sc * 256 + 128, sc * 256 + 256)
                sp = sppool.tile([128, 384], F32, name="sp")
                first = True
                for (osl, ls, rs) in (
                    (slice(0, 128), i0s, i0s),
                    (slice(128, 256), i0s, i1s),
                    (slice(256, 384), i1s, i1s),
                ):
                    for c in range(2):
                        nc.tensor.matmul(
                            sp[:, osl],
                            lhsT=qrt[h][c][:, ls],
                            rhs=qrt[h][c][:, rs],
                            start=first, stop=(c == 1),
                            skip_group_check=True,
                        )
                        first = False
                sts = stspool.tile([128, 384], FP16, name="sts")
                nc.vector.tensor_tensor(
                    out=sts, in0=sp, in1=mask[:, 0:384], op=MULT
                )
                return sts

            def emit_out(h, sc, sts, last_head=False):
                """inter + intra2 for both chunks into one O bank; state for
                both chunks; single O cast + DMA; single H copy."""
                i0c, i1c = 2 * sc, 2 * sc + 1
                m0 = slice(i0c * n, (i0c + 1) * n)
                m1 = slice(i1c * n, (i1c + 1) * n)
                i0s = slice(sc * 256, sc * 256 + 128)
                i1s = slice(sc * 256 + 128, sc * 256 + 256)
                op = opool.tile([128, 512], F32, name="op", tag="op")
                first = True
                if sc > 0:  # inter: O_i += QR_i @ H_{<superchunk}
                    for (osl, csl) in ((slice(0, 256), i0s), (slice(256, 512), i1s)):
                        for c in range(2):
                            nc.tensor.matmul(
                                op[:, osl],
                                lhsT=qrt[h][c][:, csl],
                                rhs=hs[h][:, c * 256:(c + 1) * 256],
                                start=first, stop=False,
                                skip_group_check=True,
                            )
                            first = False
                # intra2: diagonal score blocks @ V
                nc.tensor.matmul(
                    op[:, 0:256], lhsT=sts[:, 0:128], rhs=vh[h][:, m0],
                    start=first, stop=(sc > 0), skip_group_check=True,
                )
                nc.tensor.matmul(
                    op[:, 256:512], lhsT=sts[:, 128:256], rhs=vh[h][:, m0],
                    start=False, stop=False, skip_group_check=True,
                )
                nc.tensor.matmul(
                    op[:, 256:512], lhsT=sts[:, 256:384], rhs=vh[h][:, m1],
                    start=False, stop=True, skip_group_check=True,
                )
                oh = ohpool.tile([128, 512], FP16, name="oh")
                if dr[0] % 2 == 0:
                    nc.scalar.copy(out=oh, in_=op)
                else:
                    nc.vector.tensor_copy(out=oh, in_=op)
                nc.gpsimd.dma_start(
                    out=od[h][:, i0c * n:(i1c + 1) * n], in_=oh
                )
                if sc < ns - 1:
                    # state: H += QR_i0^T V_i0 + QR_i1^T V_i1 (open fp32
                    # accumulation across the head; only the head's first
                    # matmul may use start=True — bank-wide bit clear)
                    for ci, msl in ((i0c, m0), (i1c, m1)):
                        for c in range(2):
                            nc.tensor.matmul(
                                hp[h][:, c * 256:(c + 1) * 256],
                                lhsT=qtn[h][:, ci * n + c * 128: ci * n + (c + 1) * 128],
                                rhs=vh[h][:, msl],
                                start=(ci == 0 and c == 0),
                                stop=(sc == ns - 2 and ci == i1c),
                                skip_group_check=True,
                            )
                    if dr[0] % 2 == 0:
                        nc.vector.tensor_copy(out=hs[h], in_=hp[h])
                    else:
                        nc.scalar.copy(out=hs[h], in_=hp[h])
                dr[0] += 1

            for pair in range(h_per_core // 2):
                heads = (2 * pair, 2 * pair + 1)
                for h in heads:
                    emit_loads(h)
                    hp[h] = hpool.tile([128, 512], F32, tag="hp", name=f"hp{h}")
                    hs[h] = hspool.tile([128, 512], FP16, tag="hs", name=f"hs{h}")
                # superchunk 0: batch both heads' scores first so neither
                # head's intra2 waits on its own score-drain with no PE cover
                w0 = {h: emit_scores(h, 0) for h in heads}
                for h in heads:
                    emit_out(h, 0, w0[h])
                for sc in range(1, ns):
                    for h in heads:
                        sts = emit_scores(h, sc)
                        emit_out(
                            h, sc, sts,
                            last_head=(pair == h_per_core // 2 - 1 and h == heads[1]),
                        )

    if waitsplit:
        _split_overloaded_waits(nc)
    return nc


_NC_CACHE = {}


def get_nc(h_per_core=H_PER_CORE, t=T, n=N):
    key = (h_per_core, t, n)
    if key not in _NC_CACHE:
        _NC_CACHE[key] = build_nc(h_per_core, t, n)
    return _NC_CACHE[key]


def make_in_maps(Q, V, n_cores=N_CORES):
    b, nh, t, n = Q.shape
    h_per_core = (b * nh) // n_cores
    nt = t // 128
    qf = np.asarray(Q, dtype=np.float32).reshape(b * nh, t, n)
    vf = np.asarray(V, dtype=np.float32).reshape(b * nh, t, n)
    # RoPE on host in fp32 (input prep, like the layout transposes):
    # qr = q * cos + pairswap(q) * sign-folded-sin
    qsw = qf.reshape(b * nh, t, n // 2, 2)[..., ::-1].reshape(b * nh, t, n)
    cos, sin_a = rope_tables(t, n)
    qr = (qf * cos + qsw * sin_a).astype(HF)
    # (n, t) layout, n-halves split for direct 128-partition DMAs
    qrtb = np.ascontiguousarray(
        qr.transpose(0, 2, 1).reshape(b * nh, 2, 128, t)
    )

    def pmajor(x):  # [h, t, n] -> [h, 128, nt*n] with x[h, ci*128+p, m]
        return np.ascontiguousarray(
            x.reshape(b * nh, nt, 128, n).transpose(0, 2, 1, 3)
        ).reshape(b * nh, 128, nt * n)

    qtnb = pmajor(qr)
    vb = pmajor(vf.astype(HF))
    in_maps = []
    for c in range(n_cores):
        sl = slice(c * h_per_core, (c + 1) * h_per_core)
        in_maps.append(
            {
                "qrt": np.ascontiguousarray(qrtb[sl]),
                "qtn": np.ascontiguousarray(qtnb[sl]),
                "v": np.ascontiguousarray(vb[sl]),
            }
        )
    return in_maps


def unpack_out(outs, b, nh, t, n):
    """[cores][h, 128, nt*n] p-major fp16 -> (b, nh, t, n) fp32."""
    nt = t // 128
    full = np.concatenate(outs, axis=0)  # (b*nh, 128, nt*n)
    full = full.reshape(b * nh, 128, nt, n).transpose(0, 2, 1, 3)
    return np.ascontiguousarray(full).reshape(b, nh, t, n).astype(np.float32)


def kernel(Q, K, V):
    """Full-input entry point: Q, K, V are (B, NH, T, N) float32 numpy arrays.
    K is unused (the module self-keys attention on rotated Q)."""
    Q = np.asarray(Q)
    V = np.asarray(V)
    b, nh, t, n = Q.shape
    nc = get_nc((b * nh) // N_CORES, t, n)
    in_maps = make_in_maps(Q, V, N_CORES)
    res = None
    last_err = None
    for attempt in range(3):  # retry transient device/runtime failures
        try:
            res = run_bass_kernel_spmd(
                nc, in_maps, core_ids=list(range(N_CORES)), trace=False
            )
            break
        except Exception as e:  # e.g. NRT_EXEC_UNIT_UNRECOVERABLE after a
            last_err = e  # wedged prior run; a clean retry usually recovers
            import time as _time

            _time.sleep(2.0 * (attempt + 1))
    if res is None:
        raise last_err
    outs = [res.results[c]["o"] for c in range(N_CORES)]
    return unpack_out(outs, b, nh, t, n)


# revision 20
# speedup vs baseline: 1.4592x; 1.0213x over previous
"""Trainium2 Bass kernel for ContinuousAttention (self-keyed RoPE attention,
strictly-causal masked scores, no softmax).

Reference computation (B=2, NH=16, T=2048, N=256, fp32):
    QR = rope(Q)                      # interleaved-pair RoPE, freqs quantized in pairs
    S  = QR @ QR^T                    # per (b, h); K input is unused by the module
    O  = (S * strict_causal_mask) @ V

Sharding: 32 (b*nh) heads over 8 NeuronCores, 4 heads per core; no
communication.  Each core runs an identical program on its head slice.

v6 design — chunked linear attention (no softmax => scores are linear):
    O_i = QR_i @ H_{<i} + (causal diagonal blocks) @ V,   H += QR_i^T V_i
with a running state H (256x256) accumulated in fp32 PSUM across each head.
PE work is ~2*T*N^2 + ~2.5*T*C*N per head, ~2.7x less than dense-causal.

Superchunks of 256 rows (2 chunks i0, i1) keep the PSUM-drain op count low
(vector/scalar are the only engines that may read PSUM and each drain op has
a few-hundred-ns fixed cost):
  - one [128, 384] score PSUM bank holds diag(i0) | dense(i1,i0) | diag(i1),
    drained by a single mask-multiply-cast (mask = strict|ones|strict),
  - one [128, 512] O PSUM bank holds O_i0 | O_i1, drained by a single cast,
  - one H copy per superchunk; O_i1's missing chunk-i0 term comes from the
    dense block instead of H.
PSUM has_written semantics: start=True clears the accumulate bits of the
WHOLE bank, so only the first matmul targeting a bank uses it; later groups
in the same bank open with start=False (overwrite-where-unset).

Host ships QR pre-rotated in both (n, t) and p-major (t, n) fp16 layouts and
V p-major fp16; all device DMAs are contiguous 2D copies.  Output is fp16
p-major, unpacked on host.  Two heads are interleaved superchunk-by-
superchunk so every drain has a full other-head superchunk of latency cover.
"""

import math
import sys

import numpy as np

if "/opt/trn_rl_repo" not in sys.path:
    sys.path.insert(0, "/opt/trn_rl_repo")

import concourse.bass as bass
import concourse.mybir as mybir
import concourse.tile as tile
from concourse.bass_utils import run_bass_kernel_spmd

B, NH, T, N = 2, 16, 2048, 256
THETA = 2 ** 16
N_CORES = 8
H_PER_CORE = (B * NH) // N_CORES

F32 = mybir.dt.float32
FP16 = mybir.dt.float16
MULT = mybir.AluOpType.mult
HF = np.float16


def _split_overloaded_waits(nc, max_waits=1):
    """walrus in this container rejects >1 sync-wait per instruction; move
    extra waits onto preceding same-engine NoOps (semantically identical)."""
    n_split = 0
    for f in nc.m.functions:
        for bb in f.blocks:
            new_list = []
            changed = False
            for ins in bb.instructions:
                si = getattr(ins, "sync_info", None)
                if si is not None and len(si.on_wait) > max_waits:
                    waits = list(si.on_wait)
                    extra, keep = waits[:-max_waits], waits[-max_waits:]
                    k = 0
                    while extra:
                        chunk, extra = extra[:max_waits], extra[max_waits:]
                        nop = mybir.InstNoOp(
                            name=f"{ins.name}_wsplit{k}", ins=[], outs=[]
                        )
                        nop.engine = ins.engine
                        nop.sync_info = mybir.SyncInfo(on_wait=chunk, on_update=[])
                        new_list.append(nop)
                        k += 1
                    ins.sync_info = mybir.SyncInfo(
                        on_wait=keep, on_update=list(si.on_update)
                    )
                    changed = True
                    n_split += 1
                new_list.append(ins)
            if changed:
                bb.instructions = new_list
    return n_split


def rope_tables(t=T, n=N, dtype=np.float32):
    """cos table and sign-folded sin table, natural (t, n) layout."""
    idx = np.floor(np.arange(n, dtype=dtype) / dtype(2.0)) * dtype(2.0)
    freqs = (
        dtype(1.0) / (dtype(THETA) ** (idx / dtype(n))) / dtype(2.0 * math.pi)
    ).astype(dtype)
    phases = np.arange(t, dtype=dtype)[:, None] * freqs[None, :]
    ph = (phases % dtype(1.0)) * dtype(2.0 * math.pi)
    cos = np.cos(ph).astype(dtype)
    sin = np.sin(ph).astype(dtype)
    sin_a = sin.copy()
    sin_a[:, 0::2] *= dtype(-1.0)  # fold the rotate-pair sign into sin
    return cos, sin_a


def build_nc(h_per_core=H_PER_CORE, t=T, n=N, waitsplit=True):
    assert n == 256 and t % 256 == 0
    nt = t // 128   # 128-row chunks per head (16)
    ns = t // 256   # superchunks per head (8)
    nc = bass.Bass("TRN2", target_bir_lowering=False, debug=False)

    # qrt: rotated Q, (n, t) layout, two 128-partition n-halves
    qrtd = nc.dram_tensor(
        "qrt", [h_per_core, 2, 128, t], FP16, kind="ExternalInput"
    ).ap()
    # qtn: rotated Q, p-major packed (t, n): qtn[h, p, ci*n+m] = QR[h, ci*128+p, m]
    qtnd = nc.dram_tensor(
        "qtn", [h_per_core, 128, nt * n], FP16, kind="ExternalInput"
    ).ap()
    # v: p-major packed like qtn
    vd = nc.dram_tensor(
        "v", [h_per_core, 128, nt * n], FP16, kind="ExternalInput"
    ).ap()
    # o: p-major packed fp16; host unpacks + casts
    od = nc.dram_tensor(
        "o", [h_per_core, 128, nt * n], FP16, kind="ExternalOutput"
    ).ap()

    with tile.TileContext(nc) as tc:
        with (
            tc.tile_pool(name="const", bufs=1) as cpool,
            tc.tile_pool(name="qrt", bufs=4) as qpool,
            tc.tile_pool(name="qtn", bufs=4) as qnpool,
            tc.tile_pool(name="vh", bufs=4) as vpool,
            tc.tile_pool(name="hs", bufs=2) as hspool,
            tc.tile_pool(name="sts", bufs=4) as stspool,
            tc.tile_pool(name="ohs", bufs=3) as ohpool,
            tc.tile_pool(name="hp", bufs=2, space="PSUM") as hpool,
            tc.tile_pool(name="op", bufs=4, space="PSUM") as opool,
            tc.tile_pool(name="sp", bufs=2, space="PSUM") as sppool,
        ):
            # mask for one superchunk's score drain, (s, t') orientation:
            # [0:128]  = strict upper (diag i0), [128:256] = ones (dense),
            # [256:384]= strict upper (diag i1), [384:512] = ones (warmup).
            mask = cpool.tile([128, 512], F32)
            nc.gpsimd.memset(mask, 1.0)
            for c0 in (0, 256):
                nc.gpsimd.affine_select(
                    out=mask[:, c0:c0 + 128],
                    in_=mask[:, c0:c0 + 128],
                    compare_op=mybir.AluOpType.is_ge,
                    fill=0.0,
                    base=-1,
                    pattern=[[1, 128]],
                    channel_multiplier=-1,
                )

            # HAM warmup: dummy fp32 PE activity while head 0's input DMAs
            # are in flight starts the un-throttle clock early.
            for _ in range(3):
                warm = opool.tile([128, 512], F32, tag="op", name="warm")
                nc.tensor.matmul(
                    warm, lhsT=mask[:, 0:128], rhs=mask,
                    start=True, stop=True,
                )

            qrt = {}
            qtn = {}
            vh = {}
            hp = {}
            hs = {}
            dr = [0]

            def emit_loads_first(h):
                """first segment of each tensor — what superchunks 0-1 read;
                lands before the bulk so head 0 starts promptly."""
                qrt[h] = [
                    qpool.tile([128, t], FP16, tag=f"qrt{c}", name=f"qrt{c}_{h}")
                    for c in range(2)
                ]
                qtn[h] = qnpool.tile([128, nt * n], FP16, tag="qtn", name=f"qtn{h}")
                vh[h] = vpool.tile([128, nt * n], FP16, tag="vh", name=f"vh{h}")
                tsl = slice(0, t // 4)
                for c in range(2):
                    nc.sync.dma_start(out=qrt[h][c][:, tsl], in_=qrtd[h, c][:, tsl])
                vsl = slice(0, nt * n // 4)
                nc.scalar.dma_start(out=vh[h][:, vsl], in_=vd[h][:, vsl])
                nc.scalar.dma_start(out=qtn[h][:, vsl], in_=qtnd[h][:, vsl])

            def emit_loads_rest(h):
                for s in range(1, 4):
                    tsl = slice(s * (t // 4), (s + 1) * (t // 4))
                    for c in range(2):
                        nc.sync.dma_start(
                            out=qrt[h][c][:, tsl], in_=qrtd[h, c][:, tsl]
                        )
                    vsl = slice(s * (nt * n // 4), (s + 1) * (nt * n // 4))
                    nc.scalar.dma_start(out=vh[h][:, vsl], in_=vd[h][:, vsl])
                    qeng = nc.sync if s % 2 == 0 else nc.scalar
                    qeng.dma_start(out=qtn[h][:, vsl], in_=qtnd[h][:, vsl])

            def emit_scores(h, sc):
                """diag(i0) | dense(i1<-i0) | diag(i1) into one PSUM bank,
                single fused mask-mult-cast drain."""
                i0s = slice(sc * 256, sc * 256 + 128)
                i1s = slice(sc * 256 + 128, sc * 256 + 256)
                scs = slice(sc * 256, sc * 256 + 256)
                sp = sppool.tile([128, 384], F32, name="sp")
                first = True
                # diag0|dense share lhsT=QR_i0^T: one 256-wide rhs covers both
                for (osl, ls, rs) in (
                    (slice(0, 256), i0s, scs),
                    (slice(256, 384), i1s, i1s),
                ):
                    for c in range(2):
                        nc.tensor.matmul(
                            sp[:, osl],
                            lhsT=qrt[h][c][:, ls],
                            rhs=qrt[h][c][:, rs],
                            start=first, stop=(c == 1),
                            skip_group_check=True,
                        )
                        first = False
                sts = stspool.tile([128, 384], FP16, name="sts")
                nc.vector.tensor_tensor(
                    out=sts, in0=sp, in1=mask[:, 0:384], op=MULT
                )
                return sts

            def emit_out(h, sc, sts, last_head=False):
                """inter + intra2 for both chunks into one O bank; state for
                both chunks; single O cast + DMA; single H copy."""
                i0c, i1c = 2 * sc, 2 * sc + 1
                m0 = slice(i0c * n, (i0c + 1) * n)
                m1 = slice(i1c * n, (i1c + 1) * n)
                i0s = slice(sc * 256, sc * 256 + 128)
                i1s = slice(sc * 256 + 128, sc * 256 + 256)
                op = opool.tile([128, 512], F32, name="op", tag="op")
                first = True
                if sc > 0:  # inter: O_i += QR_i @ H_{<superchunk}
                    for (osl, csl) in ((slice(0, 256), i0s), (slice(256, 512), i1s)):
                        for c in range(2):
                            nc.tensor.matmul(
                                op[:, osl],
                                lhsT=qrt[h][c][:, csl],
                                rhs=hs[h][:, c * 256:(c + 1) * 256],
                                start=first, stop=False,
                                skip_group_check=True,
                            )
                            first = False
                # intra2: diagonal score blocks @ V
                nc.tensor.matmul(
                    op[:, 0:256], lhsT=sts[:, 0:128], rhs=vh[h][:, m0],
                    start=first, stop=(sc > 0), skip_group_check=True,
                )
                nc.tensor.matmul(
                    op[:, 256:512], lhsT=sts[:, 128:256], rhs=vh[h][:, m0],
                    start=False, stop=False, skip_group_check=True,
                )
                nc.tensor.matmul(
                    op[:, 256:512], lhsT=sts[:, 256:384], rhs=vh[h][:, m1],
                    start=False, stop=True, skip_group_check=True,
                )
                oh = ohpool.tile([128, 512], FP16, name="oh")
                if dr[0] % 2 == 0:
                    nc.scalar.copy(out=oh, in_=op)
                else:
                    nc.vector.tensor_copy(out=oh, in_=op)
                nc.gpsimd.dma_start(
                    out=od[h][:, i0c * n:(i1c + 1) * n], in_=oh
                )
                if sc < ns - 1:
                    # state: H += QR_i0^T V_i0 + QR_i1^T V_i1 (open fp32
                    # accumulation across the head; only the head's first
                    # matmul may use start=True — bank-wide bit clear)
                    for ci, msl in ((i0c, m0), (i1c, m1)):
                        for c in range(2):
                            nc.tensor.matmul(
                                hp[h][:, c * 256:(c + 1) * 256],
                                lhsT=qtn[h][:, ci * n + c * 128: ci * n + (c + 1) * 128],
                                rhs=vh[h][:, msl],
                                start=(ci == 0 and c == 0),
                                stop=(sc == ns - 2 and ci == i1c),
                                skip_group_check=True,
                            )
                    if dr[0] % 2 == 0:
                        nc.vector.tensor_copy(out=hs[h], in_=hp[h])
                    else:
                        nc.scalar.copy(out=hs[h], in_=hp[h])
                dr[0] += 1

            for pair in range(h_per_core // 2):
                heads = (2 * pair, 2 * pair + 1)
                for h in heads:
                    emit_loads_first(h)
                for h in heads:
                    emit_loads_rest(h)
                    hp[h] = hpool.tile([128, 512], F32, tag="hp", name=f"hp{h}")
                    hs[h] = hspool.tile([128, 512], FP16, tag="hs", name=f"hs{h}")
                # scores run one superchunk ahead of the out/state stage so
                # every score-drain has a full stage of PE cover
                cur = {h: emit_scores(h, 0) for h in heads}
                for sc in range(ns):
                    nxt = {}
                    for h in heads:
                        if sc + 1 < ns:
                            nxt[h] = emit_scores(h, sc + 1)
                        emit_out(h, sc, cur[h])
                    cur = nxt

    if waitsplit:
        _split_overloaded_waits(nc)
    return nc


_NC_CACHE = {}


def get_nc(h_per_core=H_PER_CORE, t=T, n=N):
    key = (h_per_core, t, n)
    if key not in _NC_CACHE:
        _NC_CACHE[key] = build_nc(h_per_core, t, n)
    return _NC_CACHE[key]


def make_in_maps(Q, V, n_cores=N_CORES):
    b, nh, t, n = Q.shape
    h_per_core = (b * nh) // n_cores
    nt = t // 128
    qf = np.asarray(Q, dtype=np.float32).reshape(b * nh, t, n)
    vf = np.asarray(V, dtype=np.float32).reshape(b * nh, t, n)
    # RoPE on host in fp32 (input prep, like the layout transposes):
    # qr = q * cos + pairswap(q) * sign-folded-sin
    qsw = qf.reshape(b * nh, t, n // 2, 2)[..., ::-1].reshape(b * nh, t, n)
    cos, sin_a = rope_tables(t, n)
    qr = (qf * cos + qsw * sin_a).astype(HF)
    # (n, t) layout, n-halves split for direct 128-partition DMAs
    qrtb = np.ascontiguousarray(
        qr.transpose(0, 2, 1).reshape(b * nh, 2, 128, t)
    )

    def pmajor(x):  # [h, t, n] -> [h, 128, nt*n] with x[h, ci*128+p, m]
        return np.ascontiguousarray(
            x.reshape(b * nh, nt, 128, n).transpose(0, 2, 1, 3)
        ).reshape(b * nh, 128, nt * n)

    qtnb = pmajor(qr)
    vb = pmajor(vf.astype(HF))
    in_maps = []
    for c in range(n_cores):
        sl = slice(c * h_per_core, (c + 1) * h_per_core)
        in_maps.append(
            {
                "qrt": np.ascontiguousarray(qrtb[sl]),
                "qtn": np.ascontiguousarray(qtnb[sl]),
                "v": np.ascontiguousarray(vb[sl]),
            }
        )
    return in_maps


def unpack_out(outs, b, nh, t, n):
    """[cores][h, 128, nt*n] p-major fp16 -> (b, nh, t, n) fp32."""
    nt = t // 128
    full = np.concatenate(outs, axis=0)  # (b*nh, 128, nt*n)
    full = full.reshape(b * nh, 128, nt, n).transpose(0, 2, 1, 3)
    return np.ascontiguousarray(full).reshape(b, nh, t, n).astype(np.float32)


def kernel(Q, K, V):
    """Full-input entry point: Q, K, V are (B, NH, T, N) float32 numpy arrays.
    K is unused (the module self-keys attention on rotated Q)."""
    Q = np.asarray(Q)
    V = np.asarray(V)
    b, nh, t, n = Q.shape
    nc = get_nc((b * nh) // N_CORES, t, n)
    in_maps = make_in_maps(Q, V, N_CORES)
    res = None
    last_err = None
    for attempt in range(3):  # retry transient device/runtime failures
        try:
            res = run_bass_kernel_spmd(
                nc, in_maps, core_ids=list(range(N_CORES)), trace=False
            )
            break
        except Exception as e:  # e.g. NRT_EXEC_UNIT_UNRECOVERABLE after a
            last_err = e  # wedged prior run; a clean retry usually recovers
            import time as _time

            _time.sleep(2.0 * (attempt + 1))
    if res is None:
        raise last_err
    outs = [res.results[c]["o"] for c in range(N_CORES)]
    return unpack_out(outs, b, nh, t, n)
